# revision 21
# baseline (speedup 1.0000x reference)
"""PointNet++ feature propagation kernel for Trainium2 (8 NeuronCores).

Data-parallel over batch (2 batches/core). Key techniques:
  * nd = 2*x1.x2 - |x2|^2 via one K=21 bf16 matmul per 128-row chunk:
    every fp32 factor is split hi/mid/lo into bf16 (6 product terms per
    coordinate + 3 |x2|^2 rows), streaming 1 cycle/column instead of
    fp32's 4 while keeping ~2^-27 term accuracy (top-3 selection then
    agrees with the fp32 reference on this dataset).
  * top-3 per row: vector.max8 straight from PSUM.
  * custom DVE ops: RECIP_SHIFT (1/(s1p - m8), one pass) and
    MASKED_RECIP ((dW <= t3W) * ~1/dW in one pass, bf16 out). The mask
    threshold t3W is produced by the same scalar-engine transform that
    makes dW, so the compare is bit-exact and selects exactly 3 points.
  * all W / interp / conv-weight transposes via DMA XBAR (multi-group:
    one instruction transposes up to 8 [128,128] bf16 tiles); zero PE
    identity transposes except tiny fp32 ones at setup.
  * conv0 p1-half in pure fp32 (exact); interp-half + conv1 in bf16
    hi/lo split weights (~2^-18 weight accuracy, 1 cyc/col).
  * BN stats fused into PSUM evacuation via accum_out; sum-of-squares
    on the DVE from the stored bf16 tiles.
  * y0/y1 stay in SBUF (bf16); 2-KB AllReduces for BN stats with a
    pool-queue-only warmup AllReduce absorbing collective setup.
  * software-pipelined emission: interp+conv0 of chunk q-1 are emitted
    after the dist/W-build of chunk q so the in-order PE queue never
    waits on the W transposes; batch 1 prep hides under batch 0 tails.
"""
import numpy as np
from contextlib import ExitStack

import concourse.bacc as bacc
import concourse.bass as bass
import concourse.tile as tile
import concourse.mybir as mybir
from concourse.bass_utils import run_bass_kernel_spmd

dt = mybir.dt
AF = mybir.ActivationFunctionType
ALU = mybir.AluOpType

# Problem shape (hardcoded per harness contract)
B, N, S, C1, C2 = 16, 4096, 1024, 256, 256
CIN = C1 + C2
M0, M1 = 256, 256
N_CORES = 8
BN_EPS = 1e-5

# ---- custom DVE ops ----
import concourse.dve_ops as dve_ops
from concourse.dve_spec import (C0 as DC0, C1 as DC1, C2 as DC2, AluOp, Bin,
                                Spec, Src0, Zero, select)

MR_C1 = -0.23734999999999998   # Chebyshev seed scale (tuned for 1 NR pass)
MR_C2 = 2.00225                # Newton constant; ~0.23% max rel err


def _ref_masked_recip(in0, in1, s0, s1, imm2):
    not_x = (~in0.view(np.int32)).view(np.float32)
    y0 = (not_x * np.float32(s1)).astype(np.float32)
    y1 = (y0 * (np.float32(imm2) - in0 * y0)).astype(np.float32)
    return np.where(in0 <= s0, y1, 0.0).astype(np.float32)


def _ref_recip_shift(in0, in1, s0, s1, imm2):
    t = (np.float32(s0) - in0).astype(np.float32)
    not_x = (~t.view(np.int32)).view(np.float32)
    y0 = (not_x * np.float32(s1)).astype(np.float32)
    return (y0 * (np.float32(imm2) - t * y0)).astype(np.float32)


def _register(name, spec, shas):
    op = dve_ops.DveOp(name, spec, subdim=False, uops_sha=shas)
    if op.name not in dve_ops._SUB_OPCODE_FOR_NAME:
        dve_ops.OPS.append(op)
        dve_ops.CUSTOM_DVE_SPECS[op.name] = op.spec
        dve_ops._SUB_OPCODE_FOR_NAME[op.name] = (
            dve_ops._CUSTOM_DVE_ROW_BASE + len(dve_ops.OPS) - 1)
    return op


def _make_ops():
    _not_x = Bin(AluOp.BITWISE_NOT, Src0, Src0)
    _y0 = _not_x * DC1
    _y1 = _y0 * (DC2 - Src0 * _y0)
    mr = _register(
        "MASKED_RECIP_ANT",
        Spec(body=select(Src0 <= DC0, _y1, Zero), reference=_ref_masked_recip),
        {"v3": "6144301bc1615a39", "v4": "9f6d71f7b7d4e9f5"})
    _t = DC0 - Src0
    _nt = Bin(AluOp.BITWISE_NOT, _t, _t)
    _z0 = _nt * DC1
    rs = _register(
        "RECIP_SHIFT_ANT",
        Spec(body=_z0 * (DC2 - _t * _z0), reference=_ref_recip_shift),
        {"v3": "09a91c89244bc4a0", "v4": "66b79eb6ebcf85c6"})
    return mr, rs


MASKED_RECIP, RECIP_SHIFT = _make_ops()


def build_core_kernel(nc, Bc=2, n=N, s=S, c1=C1, c2=C2, m0=M0, m1=M1,
                      replica_groups=None, use_collective=True):
    if replica_groups is None:
        replica_groups = [list(range(N_CORES))]
    world = len(replica_groups[0])
    cin = c1 + c2
    NPOS = world * Bc * n
    NC = n // 128                  # 32
    NQ = n // 512                  # 8
    SJ = s // 128                  # 8
    KP = c1 // 128                 # 2 fp32 k-chunks (p1)
    KC1 = m0 // 128                # 2
    MC0 = m0 // 128                # 2
    MC1 = m1 // 128                # 2
    C2C = c2 // 128                # 2

    d_xyz1 = nc.dram_tensor("xyz1", [Bc, 3, n], dt.float32, kind="ExternalInput")
    d_xyz2 = nc.dram_tensor("xyz2", [Bc, 3, s], dt.float32, kind="ExternalInput")
    d_p1 = nc.dram_tensor("points1", [Bc, c1, n], dt.float32, kind="ExternalInput")
    d_p2 = nc.dram_tensor("points2", [Bc, c2, s], dt.float32, kind="ExternalInput")
    d_w0 = nc.dram_tensor("w0", [m0, cin], dt.float32, kind="ExternalInput")
    d_b0 = nc.dram_tensor("b0", [m0], dt.float32, kind="ExternalInput")
    d_g0 = nc.dram_tensor("g0", [m0], dt.float32, kind="ExternalInput")
    d_be0 = nc.dram_tensor("be0", [m0], dt.float32, kind="ExternalInput")
    d_w1 = nc.dram_tensor("w1", [m1, m0], dt.float32, kind="ExternalInput")
    d_b1 = nc.dram_tensor("b1", [m1], dt.float32, kind="ExternalInput")
    d_g1 = nc.dram_tensor("g1", [m1], dt.float32, kind="ExternalInput")
    d_be1 = nc.dram_tensor("be1", [m1], dt.float32, kind="ExternalInput")
    d_out = nc.dram_tensor("out", [Bc, m1, n], dt.float32, kind="ExternalOutput")

    with tile.TileContext(nc) as tc, ExitStack() as ctx:
        consts = ctx.enter_context(tc.tile_pool(name="consts", bufs=1))
        sb = ctx.enter_context(tc.tile_pool(name="sb", bufs=2))
        sbq = ctx.enter_context(tc.tile_pool(name="sbq", bufs=2))
        psum = ctx.enter_context(tc.tile_pool(name="psum", bufs=1, space="PSUM"))
        dram = ctx.enter_context(tc.tile_pool(name="dram", bufs=1, space="DRAM"))

        # ---- persistent stores ----
        y0st = [consts.tile([128, Bc * NQ * 512], dt.bfloat16, name=f"y0st{m}")
                for m in range(MC0)]
        y1st = [consts.tile([128, Bc * NQ * 512], dt.bfloat16, name=f"y1st{m}")
                for m in range(MC1)]
        s0sum = consts.tile([128, MC0 * Bc * NQ], dt.float32)
        s0sq = consts.tile([128, MC0 * Bc * NQ], dt.float32)
        s1sum = consts.tile([128, MC1 * Bc * NQ], dt.float32)
        s1sq = consts.tile([128, MC1 * Bc * NQ], dt.float32)
        sqdump = consts.tile([128, 512], dt.float32)

        def scol(m, b, q):
            return (m * Bc + b) * NQ + q

        def load_chan_vec(dten, m):
            t = consts.tile([128, m // 128], dt.float32)
            nc.sync.dma_start(t[:], dten[:].rearrange("(c p) -> p c", p=128))
            return t

        t_b0 = load_chan_vec(d_b0, m0)
        t_g0 = load_chan_vec(d_g0, m0)
        t_be0 = load_chan_vec(d_be0, m0)
        t_b1 = load_chan_vec(d_b1, m1)
        t_g1 = load_chan_vec(d_g1, m1)
        t_be1 = load_chan_vec(d_be1, m1)

        # ---- warmup AllReduce: reduces garbage DRAM, output unused; no
        # staging DMA so no semaphore coupling with the compute queues ----
        if use_collective:
            wdr_i = dram.tile([128, 1], dt.float32, tag="warm_i")
            wdr_o = dram.tile([128, 1], dt.float32, tag="warm_o")
            nc.gpsimd.collective_compute(
                "AllReduce", ALU.add, replica_groups=replica_groups,
                ins=[wdr_i.opt()], outs=[wdr_o.opt()])

        # ---- weights ----
        ones31 = consts.tile([3, 1], dt.float32)
        nc.vector.memset(ones31[:], 1.0)
        w0p1T = consts.tile([128, KP, m0], dt.float32)
        w0xiT = consts.tile([128, 2 * C2C, m0], dt.bfloat16)
        w1T = consts.tile([128, 2 * KC1, m1], dt.bfloat16)
        ident32 = consts.tile([128, 128], dt.float32)
        from concourse.masks import make_identity
        make_identity(nc, ident32)

        def emit_weight_setup(wsb_pool):
            for mi in range(m0 // 128):
                t = wsb_pool.tile([128, cin], dt.float32, tag="wld", name="wld")
                nc.sync.dma_start(t[:], d_w0[mi * 128:(mi + 1) * 128, :])
                pt = psum.tile([128, 256], dt.float32, tag="mmc", bufs=3)
                for ki in range(KP):
                    nc.tensor.transpose(pt[:, ki * 128:(ki + 1) * 128],
                                        t[:, ki * 128:(ki + 1) * 128], ident32[:])
                for ki in range(KP):
                    nc.scalar.copy(w0p1T[:, ki, mi * 128:(mi + 1) * 128],
                                   pt[:, ki * 128:(ki + 1) * 128])
                hi = wsb_pool.tile([128, c2], dt.bfloat16, tag="whi", name="whi")
                nc.scalar.copy(hi[:], t[:, c1:cin])
                h32 = wsb_pool.tile([128, c2], dt.float32, tag="wh32", name="wh32")
                nc.scalar.copy(h32[:], hi[:])
                nc.vector.tensor_sub(t[:, c1:cin], t[:, c1:cin], h32[:])
                lo = wsb_pool.tile([128, c2], dt.bfloat16, tag="wlo", name="wlo")
                nc.scalar.copy(lo[:], t[:, c1:cin])
                nc.sync.dma_start(w0xiT[:, 0:2, mi * 128:(mi + 1) * 128], hi[:],
                                  transpose=True)
                nc.sync.dma_start(w0xiT[:, 2:4, mi * 128:(mi + 1) * 128], lo[:],
                                  transpose=True)
            for mi in range(m1 // 128):
                t = wsb_pool.tile([128, m0], dt.float32, tag="wld", name="wld")
                nc.sync.dma_start(t[:], d_w1[mi * 128:(mi + 1) * 128, :])
                hi = wsb_pool.tile([128, m0], dt.bfloat16, tag="whi", name="whi")
                nc.scalar.copy(hi[:], t[:])
                h32 = wsb_pool.tile([128, m0], dt.float32, tag="wh32", name="wh32")
                nc.scalar.copy(h32[:], hi[:])
                nc.vector.tensor_sub(t[:], t[:], h32[:])
                lo = wsb_pool.tile([128, m0], dt.bfloat16, tag="wlo", name="wlo")
                nc.scalar.copy(lo[:], t[:])
                nc.sync.dma_start(w1T[:, 0:KC1, mi * 128:(mi + 1) * 128], hi[:],
                                  transpose=True)
                nc.sync.dma_start(w1T[:, KC1:2 * KC1, mi * 128:(mi + 1) * 128],
                                  lo[:], transpose=True)

        # ================= Phase A =================
        def emit_prep(b):
            """xyz2 rows, x1 splits -> lhT, p2T, s1p for batch b."""
            x2 = sb.tile([3, s], dt.float32, tag="x2", bufs=1)
            nc.sync.dma_start(x2[:], d_xyz2[b])
            rhs21 = sb.tile([21, s], dt.bfloat16, tag="rhs21")
            scrA = sb.tile([3, s], dt.float32, tag="scrA", bufs=1)
            s2r = sb.tile([1, s], dt.float32, tag="s2r", bufs=1)
            nc.vector.tensor_mul(scrA[:], x2[:], x2[:])
            for h0 in range(0, s, 512):
                ps_s2 = psum.tile([128, 512], dt.float32, tag="mmc", bufs=3)
                nc.tensor.matmul(ps_s2[0:1, :], ones31[:], scrA[:, h0:h0 + 512],
                                 start=True, stop=True)
                nc.scalar.copy(s2r[:, h0:h0 + 512], ps_s2[0:1, :])
            nc.vector.tensor_scalar_mul(x2[:], x2[:], 2.0)  # x2 := y2
            brow = sb.tile([3, s], dt.bfloat16, tag="brow", bufs=2)
            nc.scalar.copy(brow[:], x2[:])
            nc.scalar.copy(scrA[:], brow[:])
            nc.vector.tensor_sub(x2[:], x2[:], scrA[:])
            nc.sync.dma_start(rhs21[0:3, :], brow[:])
            nc.sync.dma_start(rhs21[3:6, :], brow[:])
            nc.sync.dma_start(rhs21[6:9, :], brow[:])
            brow2 = sb.tile([3, s], dt.bfloat16, tag="brow", bufs=2)
            nc.scalar.copy(brow2[:], x2[:])
            nc.scalar.copy(scrA[:], brow2[:])
            nc.vector.tensor_sub(x2[:], x2[:], scrA[:])
            nc.sync.dma_start(rhs21[9:12, :], brow2[:])
            nc.sync.dma_start(rhs21[12:15, :], brow2[:])
            brow3 = sb.tile([3, s], dt.bfloat16, tag="brow", bufs=2)
            nc.scalar.copy(brow3[:], x2[:])
            nc.sync.dma_start(rhs21[15:18, :], brow3[:])
            for k in range(3):
                srow = sb.tile([1, s], dt.bfloat16, tag="srow", bufs=2)
                nc.scalar.copy(srow[:], s2r[:])
                nc.sync.dma_start(rhs21[18 + k:19 + k, :], srow[:])
                if k < 2:
                    nc.scalar.copy(scrA[0:1, :], srow[:])
                    nc.vector.tensor_sub(s2r[:], s2r[:], scrA[0:1, :])

            # x1 -> x1t [128, NC, 3] via PE transposes (row load + transposes)
            x1t = sb.tile([128, NC, 3], dt.float32, tag="x1t", bufs=1)
            for g0 in range(0, NC, 16):
                x1row = sb.tile([3, 16 * 128], dt.float32, tag="x1row", bufs=2)
                nc.sync.dma_start(x1row[:], d_xyz1[b][:, g0 * 128:(g0 + 16) * 128])
                ptx = psum.tile([128, 512], dt.float32, tag="mmc", bufs=3)
                for i in range(16):
                    nc.tensor.transpose(ptx[:, i * 3:i * 3 + 3],
                                        x1row[:, i * 128:(i + 1) * 128],
                                        ident32[0:3, 0:3])
                nc.scalar.copy(x1t[:, g0:g0 + 16, :], ptx[:, 0:48])
            sq1 = sb.tile([128, NC, 3], dt.float32, tag="sq1", bufs=1)
            nc.vector.tensor_mul(sq1[:], x1t[:], x1t[:])
            s1p = sb.tile([128, NC], dt.float32, tag="s1p")
            nc.vector.reduce_sum(s1p[:], sq1[:], axis=mybir.AxisListType.X)
            nc.vector.tensor_scalar_add(s1p[:], s1p[:], 1e-8)

            x1s = sb.tile([128, NC, 128], dt.bfloat16, tag="x1s", bufs=1)
            xs32a = sb.tile([128, NC, 3], dt.float32, tag="xs32a", bufs=1)
            xs32b = sb.tile([128, NC, 3], dt.float32, tag="xs32b", bufs=1)
            nc.scalar.copy(x1s[:, :, 0:3], x1t[:])
            nc.scalar.copy(xs32a[:], x1s[:, :, 0:3])
            nc.vector.tensor_sub(xs32b[:], x1t[:], xs32a[:])
            nc.scalar.copy(x1s[:, :, 3:6], xs32b[:])
            nc.scalar.copy(xs32a[:], x1s[:, :, 3:6])
            nc.vector.tensor_sub(xs32b[:], xs32b[:], xs32a[:])
            nc.scalar.copy(x1s[:, :, 6:9], xs32b[:])
            nc.scalar.copy(x1s[:, :, 9:12], x1s[:, :, 0:3])
            nc.scalar.copy(x1s[:, :, 12:15], x1s[:, :, 3:6])
            nc.scalar.copy(x1s[:, :, 15:18], x1s[:, :, 0:3])
            nc.vector.memset(x1s[:, :, 18:21], -1.0)
            lhT = sb.tile([128, NC, 128], dt.bfloat16, tag="lhT", bufs=1)
            nc.sync.dma_start(lhT[:], x1s[:].rearrange("p a b -> p (a b)"),
                              transpose=True)

            p2T = sb.tile([128, SJ, c2], dt.bfloat16, tag="p2T", bufs=1)
            for ci in range(C2C):
                p2l = sb.tile([128, s], dt.float32, tag="p2l", name="p2l", bufs=1)
                nc.sync.dma_start(p2l[:], d_p2[b, ci * 128:(ci + 1) * 128, :])
                p2b = sb.tile([128, s], dt.bfloat16, tag="p2b", name="p2b")
                nc.vector.tensor_scalar_mul(p2b[:], p2l[:], 1.0)
                nc.sync.dma_start(p2T[:, :, ci * 128:(ci + 1) * 128], p2b[:],
                                  transpose=True)
            return dict(rhs21=rhs21, lhT=lhT, p2T=p2T, s1p=s1p)

        def emit_dist(b, P, q, rp, wt):
            """dist + W build for one r-pair of chunk q."""
            if True:
                rr = (2 * rp, 2 * rp + 1)
                ps_nds, m8s = [], []
                for r in rr:
                    i = 4 * q + r
                    ps_nd = psum.tile([128, s], dt.float32, tag="nd", bufs=2)
                    for h0 in range(0, s, 512):
                        nc.tensor.matmul(ps_nd[:, h0:h0 + 512],
                                         P["lhT"][0:21, i, :],
                                         P["rhs21"][:, h0:h0 + 512],
                                         start=True, stop=True)
                    ps_nds.append(ps_nd)
                for r, ps_nd in zip(rr, ps_nds):
                    m8 = sbq.tile([128, 8], dt.float32, tag="m8", bufs=4)
                    nc.vector.max(m8[:], ps_nd[:])
                    m8s.append(m8)
                r3s = []
                for r, m8 in zip(rr, m8s):
                    i = 4 * q + r
                    r3 = sbq.tile([128, 3], dt.float32, tag="r3", bufs=4)
                    nc.vector._custom_dve(RECIP_SHIFT, out=r3[:], in0=m8[:, 0:3],
                                          s0=P["s1p"][:, i:i + 1], s1=MR_C1,
                                          imm2=MR_C2)
                    r3s.append(r3)
                dens, negdens = [], []
                for r, r3 in zip(rr, r3s):
                    den = sbq.tile([128, 1], dt.float32, tag="den", bufs=4)
                    nc.vector.reduce_sum(den[:], r3[:], axis=mybir.AxisListType.X)
                    dens.append(den)
                for r, den in zip(rr, dens):
                    negden = sbq.tile([128, 1], dt.float32, tag="negden", bufs=4)
                    nc.vector.tensor_scalar_mul(negden[:], den[:], -1.0)
                    negdens.append(negden)
                bias2s = []
                for r, den in zip(rr, dens):
                    i = 4 * q + r
                    bias2 = sbq.tile([128, 1], dt.float32, tag="bias2", bufs=4)
                    nc.scalar.activation(bias2[:], den[:], AF.Identity,
                                         bias=0.0, scale=P["s1p"][:, i:i + 1])
                    bias2s.append(bias2)
                dWs, t8Ws = [], []
                for k, r in enumerate(rr):
                    dW = sbq.tile([128, s], dt.float32, tag="dW", bufs=2)
                    nc.scalar.activation(dW[:], ps_nds[k][:], AF.Identity,
                                         bias=bias2s[k][:, 0:1],
                                         scale=negdens[k][:, 0:1])
                    t8W = sbq.tile([128, 8], dt.float32, tag="t8W", bufs=4)
                    nc.scalar.activation(t8W[:], m8s[k][:], AF.Identity,
                                         bias=bias2s[k][:, 0:1],
                                         scale=negdens[k][:, 0:1])
                    dWs.append(dW)
                    t8Ws.append(t8W)
                for k, r in enumerate(rr):
                    wrow = sbq.tile([128, s], dt.bfloat16, tag="wrow", bufs=3)
                    nc.vector._custom_dve(MASKED_RECIP, out=wrow[:],
                                          in0=dWs[k][:], s0=t8Ws[k][:, 2:3],
                                          s1=MR_C1, imm2=MR_C2)
                    nc.sync.dma_start(wt[:, :, r * 128:(r + 1) * 128], wrow[:],
                                      transpose=True)

        def emit_interp(b, P, q, wt):
            xI = []
            for m in range(C2C):
                ps_i = psum.tile([128, 512], dt.float32, tag="mmc", bufs=3)
                for j in range(SJ):
                    nc.tensor.matmul(ps_i[:], P["p2T"][:, j, m * 128:(m + 1) * 128],
                                     wt[:, j, :], start=(j == 0),
                                     stop=(j == SJ - 1))
                t = sbq.tile([128, 512], dt.bfloat16, tag=f"xI{m}")
                nc.scalar.copy(t[:], ps_i[:])
                xI.append(t)
            return xI

        def emit_conv0(b, P, q, xI):
            p1r = []
            for m in range(KP):
                t0 = sbq.tile([128, 512], dt.float32, tag="p1l", bufs=3)
                nc.sync.dma_start(t0[:], d_p1[b, m * 128:(m + 1) * 128,
                                               q * 512:(q + 1) * 512])
                p1r.append(t0)
            col = (b * NQ + q) * 512
            for m in range(MC0):
                ps_c = psum.tile([128, 512], dt.float32, tag="mmc", bufs=3)
                nk = KP + 4
                kk = 0
                for ki in range(KP):
                    nc.tensor.matmul(ps_c[:], w0p1T[:, ki, m * 128:(m + 1) * 128],
                                     p1r[ki][:], start=(kk == 0),
                                     stop=(kk == nk - 1))
                    kk += 1
                for g in range(4):
                    nc.tensor.matmul(ps_c[:], w0xiT[:, g, m * 128:(m + 1) * 128],
                                     xI[g % C2C][:], start=(kk == 0),
                                     stop=(kk == nk - 1))
                    kk += 1
                sc = scol(m, b, q)
                nc.scalar.activation(y0st[m][:, col:col + 512], ps_c[:],
                                     AF.Identity, bias=t_b0[:, m:m + 1],
                                     accum_out=s0sum[:, sc:sc + 1])
                nc.vector.scalar_tensor_tensor(
                    out=sqdump[:], in0=y0st[m][:, col:col + 512], scalar=1.0,
                    in1=y0st[m][:, col:col + 512], op0=ALU.mult, op1=ALU.mult,
                    accum_out=s0sq[:, sc:sc + 1])

        preps = {0: emit_prep(0)}
        with tc.tile_pool(name="wsetup", bufs=1) as wsb_pool:
            emit_weight_setup(wsb_pool)
        pending = None       # (b, P, q, wt) whose interp/conv0 are outstanding
        pend_xi = None
        for b in range(Bc):
            for q in range(NQ):
                wt = sbq.tile([128, SJ, 512], dt.bfloat16, tag="wt")
                emit_dist(b, preps[b], q, 0, wt)
                if pending is not None:
                    pend_xi = emit_interp(pending[0], pending[1], pending[2],
                                          pending[3])
                emit_dist(b, preps[b], q, 1, wt)
                if pending is not None:
                    emit_conv0(pending[0], pending[1], pending[2], pend_xi)
                pending = (b, preps[b], q, wt)
                if b == 0 and q == NQ - 1:
                    preps[1] = emit_prep(1)
        pend_xi = emit_interp(*pending)
        emit_conv0(pending[0], pending[1], pending[2], pend_xi)

        # ================= BN stats AllReduce =================
        def bn_allreduce(ssum, ssq, mc, tag):
            loc = consts.tile([128, 2 * mc], dt.float32, tag=f"loc_{tag}")
            for m in range(mc):
                nc.vector.reduce_sum(loc[:, m:m + 1],
                                     ssum[:, m * Bc * NQ:(m + 1) * Bc * NQ],
                                     axis=mybir.AxisListType.X)
                nc.vector.reduce_sum(loc[:, mc + m:mc + m + 1],
                                     ssq[:, m * Bc * NQ:(m + 1) * Bc * NQ],
                                     axis=mybir.AxisListType.X)
            if not use_collective:
                return loc
            dr_in = dram.tile([128, 2 * mc], dt.float32, tag=f"cc_in_{tag}")
            dr_out = dram.tile([128, 2 * mc], dt.float32, tag=f"cc_out_{tag}")
            nc.gpsimd.dma_start(dr_in[:], loc[:])
            nc.gpsimd.collective_compute(
                "AllReduce", ALU.add, replica_groups=replica_groups,
                ins=[dr_in.opt()], outs=[dr_out.opt()])
            glob = consts.tile([128, 2 * mc], dt.float32, tag=f"glob_{tag}")
            nc.sync.dma_start(glob[:], dr_out[:])
            return glob

        def bn_coeffs(glob, mc, t_g, t_be, tag):
            mean = consts.tile([128, mc], dt.float32, tag=f"mean_{tag}")
            nc.vector.tensor_scalar_mul(mean[:], glob[:, 0:mc], 1.0 / NPOS)
            ex2 = consts.tile([128, mc], dt.float32, tag=f"ex2_{tag}")
            nc.vector.tensor_scalar_mul(ex2[:], glob[:, mc:2 * mc], 1.0 / NPOS)
            var = consts.tile([128, mc], dt.float32, tag=f"var_{tag}")
            nc.vector.tensor_mul(var[:], mean[:], mean[:])
            nc.vector.tensor_sub(var[:], ex2[:], var[:])
            std = consts.tile([128, mc], dt.float32, tag=f"std_{tag}")
            nc.vector.tensor_scalar_add(var[:], var[:], BN_EPS)
            nc.scalar.sqrt(std[:], var[:])
            rstd = consts.tile([128, mc], dt.float32, tag=f"rstd_{tag}")
            nc.vector.reciprocal(rstd[:], std[:])
            A = consts.tile([128, mc], dt.float32, tag=f"A_{tag}")
            nc.vector.tensor_mul(A[:], t_g[:], rstd[:])
            Bsh = consts.tile([128, mc], dt.float32, tag=f"B_{tag}")
            nc.vector.tensor_mul(Bsh[:], mean[:], A[:])
            nc.vector.tensor_sub(Bsh[:], t_be[:], Bsh[:])
            return A, Bsh

        glob1 = bn_allreduce(s0sum, s0sq, MC0, "l1")
        A1, B1 = bn_coeffs(glob1, MC0, t_g0, t_be0, "l1")

        # ================= Phase B: BN1+ReLU -> conv1 =================
        QG = 2  # q-group width for the BN apply
        for b in range(Bc):
            for qg in range(0, NQ, QG):
                gcol = (b * NQ + qg) * 512
                a0g = []
                for m in range(MC0):
                    t = sbq.tile([128, QG * 512], dt.bfloat16, tag=f"a0_{m}")
                    nc.scalar.activation(t[:], y0st[m][:, gcol:gcol + QG * 512],
                                         AF.Relu, bias=B1[:, m:m + 1],
                                         scale=A1[:, m:m + 1])
                    a0g.append(t)
                for qi in range(QG):
                    q = qg + qi
                    col = (b * NQ + q) * 512
                    for m in range(MC1):
                        ps_c = psum.tile([128, 512], dt.float32, tag="mmc", bufs=3)
                        for k in range(2 * KC1):
                            nc.tensor.matmul(
                                ps_c[:], w1T[:, k, m * 128:(m + 1) * 128],
                                a0g[k % KC1][:, qi * 512:(qi + 1) * 512],
                                start=(k == 0), stop=(k == 2 * KC1 - 1))
                        sc = scol(m, b, q)
                        nc.scalar.activation(y1st[m][:, col:col + 512], ps_c[:],
                                             AF.Identity, bias=t_b1[:, m:m + 1],
                                             accum_out=s1sum[:, sc:sc + 1])
                        nc.vector.scalar_tensor_tensor(
                            out=sqdump[:], in0=y1st[m][:, col:col + 512],
                            scalar=1.0, in1=y1st[m][:, col:col + 512],
                            op0=ALU.mult, op1=ALU.mult,
                            accum_out=s1sq[:, sc:sc + 1])

        glob2 = bn_allreduce(s1sum, s1sq, MC1, "l2")
        A2, B2 = bn_coeffs(glob2, MC1, t_g1, t_be1, "l2")

        # ================= Phase C: BN2+ReLU -> out =================
        for b in range(Bc):
            for qg in range(0, NQ, QG):
                gcol = (b * NQ + qg) * 512
                for m in range(MC1):
                    t = sbq.tile([128, QG * 512], dt.float32, tag="outt")
                    nc.scalar.activation(t[:], y1st[m][:, gcol:gcol + QG * 512],
                                         AF.Relu, bias=B2[:, m:m + 1],
                                         scale=A2[:, m:m + 1])
                    half = QG * 512 // 2
                    nc.sync.dma_start(d_out[b, m * 128:(m + 1) * 128,
                                            qg * 512:qg * 512 + half],
                                      t[:, 0:half])
                    nc.scalar.dma_start(d_out[b, m * 128:(m + 1) * 128,
                                              qg * 512 + half:(qg + QG) * 512],
                                        t[:, half:QG * 512])

    return nc


_CACHED = {}


def _get_compiled(key, **kw):
    if key not in _CACHED:
        nc = bacc.Bacc()
        build_core_kernel(nc, **kw)
        nc.compile()
        _CACHED[key] = nc
    return _CACHED[key]


def kernel(xyz1, xyz2, points1, points2, w0, b0, g0, be0, w1, b1, g1, be1,
           trace=False):
    xyz1 = np.ascontiguousarray(xyz1, dtype=np.float32)
    xyz2 = np.ascontiguousarray(xyz2, dtype=np.float32)
    points1 = np.ascontiguousarray(points1, dtype=np.float32)
    points2 = np.ascontiguousarray(points2, dtype=np.float32)
    shared = {
        "w0": np.ascontiguousarray(w0, dtype=np.float32),
        "b0": np.ascontiguousarray(b0, dtype=np.float32),
        "g0": np.ascontiguousarray(g0, dtype=np.float32),
        "be0": np.ascontiguousarray(be0, dtype=np.float32),
        "w1": np.ascontiguousarray(w1, dtype=np.float32),
        "b1": np.ascontiguousarray(b1, dtype=np.float32),
        "g1": np.ascontiguousarray(g1, dtype=np.float32),
        "be1": np.ascontiguousarray(be1, dtype=np.float32),
    }
    Bc = B // N_CORES
    nc = _get_compiled("full")
    in_maps = []
    for c in range(N_CORES):
        sl = slice(c * Bc, (c + 1) * Bc)
        in_maps.append({
            "xyz1": xyz1[sl], "xyz2": xyz2[sl],
            "points1": points1[sl], "points2": points2[sl],
            **shared,
        })
    res = run_bass_kernel_spmd(nc, in_maps, core_ids=list(range(N_CORES)),
                               trace=trace)
    out = np.empty((B, M1, N), dtype=np.float32)
    for c in range(N_CORES):
        out[c * Bc:(c + 1) * Bc] = res.results[c]["out"]
    if trace:
        return out, res
    return out


# revision 22
# speedup vs baseline: 1.0085x; 1.0085x over previous
"""PointNet++ feature propagation kernel for Trainium2 (8 NeuronCores).

Data-parallel over batch (2 batches/core). Key techniques:
  * nd = 2*x1.x2 - |x2|^2 via one K=21 bf16 matmul per 128-row chunk:
    every fp32 factor is split hi/mid/lo into bf16 (6 product terms per
    coordinate + 3 |x2|^2 rows), streaming 1 cycle/column instead of
    fp32's 4 while keeping ~2^-27 term accuracy (top-3 selection then
    agrees with the fp32 reference on this dataset).
  * top-3 per row: vector.max8 straight from PSUM.
  * custom DVE ops: RECIP_SHIFT (1/(s1p - m8), one pass) and
    MASKED_RECIP ((dW <= t3W) * ~1/dW in one pass, bf16 out). The mask
    threshold t3W is produced by the same scalar-engine transform that
    makes dW, so the compare is bit-exact and selects exactly 3 points.
  * all W / interp / conv-weight transposes via DMA XBAR (multi-group:
    one instruction transposes up to 8 [128,128] bf16 tiles); zero PE
    identity transposes except tiny fp32 ones at setup.
  * conv0 p1-half in pure fp32 (exact); interp-half + conv1 in bf16
    hi/lo split weights (~2^-18 weight accuracy, 1 cyc/col).
  * BN stats fused into PSUM evacuation via accum_out; sum-of-squares
    on the DVE from the stored bf16 tiles.
  * y0/y1 stay in SBUF (bf16); 2-KB AllReduces for BN stats with a
    pool-queue-only warmup AllReduce absorbing collective setup.
  * software-pipelined emission: interp+conv0 of chunk q-1 are emitted
    after the dist/W-build of chunk q so the in-order PE queue never
    waits on the W transposes; batch 1 prep hides under batch 0 tails.
"""
import numpy as np
from contextlib import ExitStack

import concourse.bacc as bacc
import concourse.bass as bass
import concourse.tile as tile
import concourse.mybir as mybir
from concourse.bass_utils import run_bass_kernel_spmd

dt = mybir.dt
AF = mybir.ActivationFunctionType
ALU = mybir.AluOpType

# Problem shape (hardcoded per harness contract)
B, N, S, C1, C2 = 16, 4096, 1024, 256, 256
CIN = C1 + C2
M0, M1 = 256, 256
N_CORES = 8
BN_EPS = 1e-5

# ---- custom DVE ops ----
import concourse.dve_ops as dve_ops
from concourse.dve_spec import (C0 as DC0, C1 as DC1, C2 as DC2, AluOp, Bin,
                                Spec, Src0, Zero, select)

MR_C1 = -0.23734999999999998   # Chebyshev seed scale (tuned for 1 NR pass)
MR_C2 = 2.00225                # Newton constant; ~0.23% max rel err


def _ref_masked_recip(in0, in1, s0, s1, imm2):
    not_x = (~in0.view(np.int32)).view(np.float32)
    y0 = (not_x * np.float32(s1)).astype(np.float32)
    y1 = (y0 * (np.float32(imm2) - in0 * y0)).astype(np.float32)
    return np.where(in0 <= s0, y1, 0.0).astype(np.float32)


def _ref_recip_shift(in0, in1, s0, s1, imm2):
    t = (np.float32(s0) - in0).astype(np.float32)
    not_x = (~t.view(np.int32)).view(np.float32)
    y0 = (not_x * np.float32(s1)).astype(np.float32)
    return (y0 * (np.float32(imm2) - t * y0)).astype(np.float32)


def _register(name, spec, shas):
    op = dve_ops.DveOp(name, spec, subdim=False, uops_sha=shas)
    if op.name not in dve_ops._SUB_OPCODE_FOR_NAME:
        dve_ops.OPS.append(op)
        dve_ops.CUSTOM_DVE_SPECS[op.name] = op.spec
        dve_ops._SUB_OPCODE_FOR_NAME[op.name] = (
            dve_ops._CUSTOM_DVE_ROW_BASE + len(dve_ops.OPS) - 1)
    return op


def _make_ops():
    _not_x = Bin(AluOp.BITWISE_NOT, Src0, Src0)
    _y0 = _not_x * DC1
    _y1 = _y0 * (DC2 - Src0 * _y0)
    mr = _register(
        "MASKED_RECIP_ANT",
        Spec(body=select(Src0 <= DC0, _y1, Zero), reference=_ref_masked_recip),
        {"v3": "6144301bc1615a39", "v4": "9f6d71f7b7d4e9f5"})
    _t = DC0 - Src0
    _nt = Bin(AluOp.BITWISE_NOT, _t, _t)
    _z0 = _nt * DC1
    rs = _register(
        "RECIP_SHIFT_ANT",
        Spec(body=_z0 * (DC2 - _t * _z0), reference=_ref_recip_shift),
        {"v3": "09a91c89244bc4a0", "v4": "66b79eb6ebcf85c6"})
    return mr, rs


MASKED_RECIP, RECIP_SHIFT = _make_ops()


def build_core_kernel(nc, Bc=2, n=N, s=S, c1=C1, c2=C2, m0=M0, m1=M1,
                      replica_groups=None, use_collective=True):
    if replica_groups is None:
        replica_groups = [list(range(N_CORES))]
    world = len(replica_groups[0])
    cin = c1 + c2
    NPOS = world * Bc * n
    NC = n // 128                  # 32
    NQ = n // 512                  # 8
    SJ = s // 128                  # 8
    KP = c1 // 128                 # 2 fp32 k-chunks (p1)
    KC1 = m0 // 128                # 2
    MC0 = m0 // 128                # 2
    MC1 = m1 // 128                # 2
    C2C = c2 // 128                # 2

    d_xyz1 = nc.dram_tensor("xyz1", [Bc, 3, n], dt.float32, kind="ExternalInput")
    d_xyz2 = nc.dram_tensor("xyz2", [Bc, 3, s], dt.float32, kind="ExternalInput")
    d_p1 = nc.dram_tensor("points1", [Bc, c1, n], dt.float32, kind="ExternalInput")
    d_p2 = nc.dram_tensor("points2", [Bc, c2, s], dt.float32, kind="ExternalInput")
    d_w0 = nc.dram_tensor("w0", [m0, cin], dt.float32, kind="ExternalInput")
    d_b0 = nc.dram_tensor("b0", [m0], dt.float32, kind="ExternalInput")
    d_g0 = nc.dram_tensor("g0", [m0], dt.float32, kind="ExternalInput")
    d_be0 = nc.dram_tensor("be0", [m0], dt.float32, kind="ExternalInput")
    d_w1 = nc.dram_tensor("w1", [m1, m0], dt.float32, kind="ExternalInput")
    d_b1 = nc.dram_tensor("b1", [m1], dt.float32, kind="ExternalInput")
    d_g1 = nc.dram_tensor("g1", [m1], dt.float32, kind="ExternalInput")
    d_be1 = nc.dram_tensor("be1", [m1], dt.float32, kind="ExternalInput")
    d_out = nc.dram_tensor("out", [Bc, m1, n], dt.float32, kind="ExternalOutput")

    with tile.TileContext(nc) as tc, ExitStack() as ctx:
        consts = ctx.enter_context(tc.tile_pool(name="consts", bufs=1))
        sb = ctx.enter_context(tc.tile_pool(name="sb", bufs=2))
        sbq = ctx.enter_context(tc.tile_pool(name="sbq", bufs=2))
        psum = ctx.enter_context(tc.tile_pool(name="psum", bufs=1, space="PSUM"))
        dram = ctx.enter_context(tc.tile_pool(name="dram", bufs=1, space="DRAM"))

        # ---- persistent stores ----
        y0st = [consts.tile([128, Bc * NQ * 512], dt.bfloat16, name=f"y0st{m}")
                for m in range(MC0)]
        y1st = [consts.tile([128, Bc * NQ * 512], dt.bfloat16, name=f"y1st{m}")
                for m in range(MC1)]
        s0sum = consts.tile([128, MC0 * Bc * NQ], dt.float32)
        s0sq = consts.tile([128, MC0 * Bc * NQ], dt.float32)
        s1sum = consts.tile([128, MC1 * Bc * NQ], dt.float32)
        s1sq = consts.tile([128, MC1 * Bc * NQ], dt.float32)
        sqdump = consts.tile([128, 512], dt.float32)

        def scol(m, b, q):
            return (m * Bc + b) * NQ + q

        def load_chan_vec(dten, m):
            t = consts.tile([128, m // 128], dt.float32)
            nc.sync.dma_start(t[:], dten[:].rearrange("(c p) -> p c", p=128))
            return t

        t_b0 = load_chan_vec(d_b0, m0)
        t_g0 = load_chan_vec(d_g0, m0)
        t_be0 = load_chan_vec(d_be0, m0)
        t_b1 = load_chan_vec(d_b1, m1)
        t_g1 = load_chan_vec(d_g1, m1)
        t_be1 = load_chan_vec(d_be1, m1)

        # ---- warmup AllReduce: reduces garbage DRAM, output unused; no
        # staging DMA so no semaphore coupling with the compute queues ----
        if use_collective:
            wdr_i = dram.tile([128, 1], dt.float32, tag="warm_i")
            wdr_o = dram.tile([128, 1], dt.float32, tag="warm_o")
            nc.gpsimd.collective_compute(
                "AllReduce", ALU.add, replica_groups=replica_groups,
                ins=[wdr_i.opt()], outs=[wdr_o.opt()])

        # ---- weights ----
        ones31 = consts.tile([3, 1], dt.float32)
        nc.vector.memset(ones31[:], 1.0)
        w0p1T = consts.tile([128, KP, m0], dt.float32)
        w0xiT = consts.tile([128, 2 * C2C, m0], dt.bfloat16)
        w1T = consts.tile([128, 2 * KC1, m1], dt.bfloat16)
        ident32 = consts.tile([128, 128], dt.float32)
        from concourse.masks import make_identity
        make_identity(nc, ident32)

        def emit_weight_setup(wsb_pool):
            for mi in range(m0 // 128):
                t = wsb_pool.tile([128, cin], dt.float32, tag="wld", name="wld")
                nc.sync.dma_start(t[:], d_w0[mi * 128:(mi + 1) * 128, :])
                pt = psum.tile([128, 256], dt.float32, tag="mmc", bufs=2)
                for ki in range(KP):
                    nc.tensor.transpose(pt[:, ki * 128:(ki + 1) * 128],
                                        t[:, ki * 128:(ki + 1) * 128], ident32[:])
                for ki in range(KP):
                    nc.scalar.copy(w0p1T[:, ki, mi * 128:(mi + 1) * 128],
                                   pt[:, ki * 128:(ki + 1) * 128])
                hi = wsb_pool.tile([128, c2], dt.bfloat16, tag="whi", name="whi")
                nc.scalar.copy(hi[:], t[:, c1:cin])
                h32 = wsb_pool.tile([128, c2], dt.float32, tag="wh32", name="wh32")
                nc.scalar.copy(h32[:], hi[:])
                nc.vector.tensor_sub(t[:, c1:cin], t[:, c1:cin], h32[:])
                lo = wsb_pool.tile([128, c2], dt.bfloat16, tag="wlo", name="wlo")
                nc.scalar.copy(lo[:], t[:, c1:cin])
                nc.sync.dma_start(w0xiT[:, 0:2, mi * 128:(mi + 1) * 128], hi[:],
                                  transpose=True)
                nc.sync.dma_start(w0xiT[:, 2:4, mi * 128:(mi + 1) * 128], lo[:],
                                  transpose=True)
            for mi in range(m1 // 128):
                t = wsb_pool.tile([128, m0], dt.float32, tag="wld", name="wld")
                nc.sync.dma_start(t[:], d_w1[mi * 128:(mi + 1) * 128, :])
                hi = wsb_pool.tile([128, m0], dt.bfloat16, tag="whi", name="whi")
                nc.scalar.copy(hi[:], t[:])
                h32 = wsb_pool.tile([128, m0], dt.float32, tag="wh32", name="wh32")
                nc.scalar.copy(h32[:], hi[:])
                nc.vector.tensor_sub(t[:], t[:], h32[:])
                lo = wsb_pool.tile([128, m0], dt.bfloat16, tag="wlo", name="wlo")
                nc.scalar.copy(lo[:], t[:])
                nc.sync.dma_start(w1T[:, 0:KC1, mi * 128:(mi + 1) * 128], hi[:],
                                  transpose=True)
                nc.sync.dma_start(w1T[:, KC1:2 * KC1, mi * 128:(mi + 1) * 128],
                                  lo[:], transpose=True)

        # ================= Phase A =================
        def emit_prep(b):
            """xyz2 rows, x1 splits -> lhT, p2T, s1p for batch b."""
            x2 = sb.tile([3, s], dt.float32, tag="x2", bufs=1)
            nc.sync.dma_start(x2[:], d_xyz2[b])
            rhs21 = sb.tile([21, s], dt.bfloat16, tag="rhs21")
            scrA = sb.tile([3, s], dt.float32, tag="scrA", bufs=1)
            s2r = sb.tile([1, s], dt.float32, tag="s2r", bufs=1)
            nc.vector.tensor_mul(scrA[:], x2[:], x2[:])
            for h0 in range(0, s, 512):
                ps_s2 = psum.tile([128, 512], dt.float32, tag="mmc", bufs=2)
                nc.tensor.matmul(ps_s2[0:1, :], ones31[:], scrA[:, h0:h0 + 512],
                                 start=True, stop=True)
                nc.scalar.copy(s2r[:, h0:h0 + 512], ps_s2[0:1, :])
            nc.vector.tensor_scalar_mul(x2[:], x2[:], 2.0)  # x2 := y2
            brow = sb.tile([3, s], dt.bfloat16, tag="brow", bufs=2)
            nc.scalar.copy(brow[:], x2[:])
            nc.scalar.copy(scrA[:], brow[:])
            nc.vector.tensor_sub(x2[:], x2[:], scrA[:])
            nc.sync.dma_start(rhs21[0:3, :], brow[:])
            nc.sync.dma_start(rhs21[3:6, :], brow[:])
            nc.sync.dma_start(rhs21[6:9, :], brow[:])
            brow2 = sb.tile([3, s], dt.bfloat16, tag="brow", bufs=2)
            nc.scalar.copy(brow2[:], x2[:])
            nc.scalar.copy(scrA[:], brow2[:])
            nc.vector.tensor_sub(x2[:], x2[:], scrA[:])
            nc.sync.dma_start(rhs21[9:12, :], brow2[:])
            nc.sync.dma_start(rhs21[12:15, :], brow2[:])
            brow3 = sb.tile([3, s], dt.bfloat16, tag="brow", bufs=2)
            nc.scalar.copy(brow3[:], x2[:])
            nc.sync.dma_start(rhs21[15:18, :], brow3[:])
            for k in range(3):
                srow = sb.tile([1, s], dt.bfloat16, tag="srow", bufs=2)
                nc.scalar.copy(srow[:], s2r[:])
                nc.sync.dma_start(rhs21[18 + k:19 + k, :], srow[:])
                if k < 2:
                    nc.scalar.copy(scrA[0:1, :], srow[:])
                    nc.vector.tensor_sub(s2r[:], s2r[:], scrA[0:1, :])

            # x1 -> x1t [128, NC, 3] via PE transposes (row load + transposes)
            x1t = sb.tile([128, NC, 3], dt.float32, tag="x1t", bufs=1)
            for g0 in range(0, NC, 16):
                x1row = sb.tile([3, 16 * 128], dt.float32, tag="x1row", bufs=2)
                nc.sync.dma_start(x1row[:], d_xyz1[b][:, g0 * 128:(g0 + 16) * 128])
                ptx = psum.tile([128, 512], dt.float32, tag="mmc", bufs=2)
                for i in range(16):
                    nc.tensor.transpose(ptx[:, i * 3:i * 3 + 3],
                                        x1row[:, i * 128:(i + 1) * 128],
                                        ident32[0:3, 0:3])
                nc.scalar.copy(x1t[:, g0:g0 + 16, :], ptx[:, 0:48])
            sq1 = sb.tile([128, NC, 3], dt.float32, tag="sq1", bufs=1)
            nc.vector.tensor_mul(sq1[:], x1t[:], x1t[:])
            s1p = sb.tile([128, NC], dt.float32, tag="s1p")
            nc.vector.reduce_sum(s1p[:], sq1[:], axis=mybir.AxisListType.X)
            nc.vector.tensor_scalar_add(s1p[:], s1p[:], 1e-8)

            x1s = sb.tile([128, NC, 128], dt.bfloat16, tag="x1s", bufs=1)
            xs32a = sb.tile([128, NC, 3], dt.float32, tag="xs32a", bufs=1)
            xs32b = sb.tile([128, NC, 3], dt.float32, tag="xs32b", bufs=1)
            nc.scalar.copy(x1s[:, :, 0:3], x1t[:])
            nc.scalar.copy(xs32a[:], x1s[:, :, 0:3])
            nc.vector.tensor_sub(xs32b[:], x1t[:], xs32a[:])
            nc.scalar.copy(x1s[:, :, 3:6], xs32b[:])
            nc.scalar.copy(xs32a[:], x1s[:, :, 3:6])
            nc.vector.tensor_sub(xs32b[:], xs32b[:], xs32a[:])
            nc.scalar.copy(x1s[:, :, 6:9], xs32b[:])
            nc.scalar.copy(x1s[:, :, 9:12], x1s[:, :, 0:3])
            nc.scalar.copy(x1s[:, :, 12:15], x1s[:, :, 3:6])
            nc.scalar.copy(x1s[:, :, 15:18], x1s[:, :, 0:3])
            nc.vector.memset(x1s[:, :, 18:21], -1.0)
            lhT = sb.tile([128, NC, 128], dt.bfloat16, tag="lhT", bufs=1)
            nc.sync.dma_start(lhT[:], x1s[:].rearrange("p a b -> p (a b)"),
                              transpose=True)

            p2T = sb.tile([128, SJ, c2], dt.bfloat16, tag="p2T", bufs=1)
            for ci in range(C2C):
                p2l = sb.tile([128, s], dt.float32, tag="p2l", name="p2l", bufs=1)
                nc.sync.dma_start(p2l[:], d_p2[b, ci * 128:(ci + 1) * 128, :])
                p2b = sb.tile([128, s], dt.bfloat16, tag="p2b", name="p2b")
                nc.vector.tensor_scalar_mul(p2b[:], p2l[:], 1.0)
                nc.sync.dma_start(p2T[:, :, ci * 128:(ci + 1) * 128], p2b[:],
                                  transpose=True)
            return dict(rhs21=rhs21, lhT=lhT, p2T=p2T, s1p=s1p)

        def emit_dist(b, P, q, rp, wt):
            """dist + W build for one r-pair of chunk q."""
            if True:
                rr = (2 * rp, 2 * rp + 1)
                ps_nds, m8s = [], []
                for r in rr:
                    i = 4 * q + r
                    ps_nd = psum.tile([128, s], dt.float32, tag="nd", bufs=3)
                    for h0 in range(0, s, 512):
                        nc.tensor.matmul(ps_nd[:, h0:h0 + 512],
                                         P["lhT"][0:21, i, :],
                                         P["rhs21"][:, h0:h0 + 512],
                                         start=True, stop=True)
                    ps_nds.append(ps_nd)
                for r, ps_nd in zip(rr, ps_nds):
                    m8 = sbq.tile([128, 8], dt.float32, tag="m8", bufs=4)
                    nc.vector.max(m8[:], ps_nd[:])
                    m8s.append(m8)
                r3s = []
                for r, m8 in zip(rr, m8s):
                    i = 4 * q + r
                    r3 = sbq.tile([128, 3], dt.float32, tag="r3", bufs=4)
                    nc.vector._custom_dve(RECIP_SHIFT, out=r3[:], in0=m8[:, 0:3],
                                          s0=P["s1p"][:, i:i + 1], s1=MR_C1,
                                          imm2=MR_C2)
                    r3s.append(r3)
                dens, negdens = [], []
                for r, r3 in zip(rr, r3s):
                    den = sbq.tile([128, 1], dt.float32, tag="den", bufs=4)
                    nc.vector.reduce_sum(den[:], r3[:], axis=mybir.AxisListType.X)
                    dens.append(den)
                for r, den in zip(rr, dens):
                    negden = sbq.tile([128, 1], dt.float32, tag="negden", bufs=4)
                    nc.vector.tensor_scalar_mul(negden[:], den[:], -1.0)
                    negdens.append(negden)
                bias2s = []
                for r, den in zip(rr, dens):
                    i = 4 * q + r
                    bias2 = sbq.tile([128, 1], dt.float32, tag="bias2", bufs=4)
                    nc.scalar.activation(bias2[:], den[:], AF.Identity,
                                         bias=0.0, scale=P["s1p"][:, i:i + 1])
                    bias2s.append(bias2)
                dWs, t8Ws = [], []
                for k, r in enumerate(rr):
                    dW = sbq.tile([128, s], dt.float32, tag="dW", bufs=2)
                    nc.scalar.activation(dW[:], ps_nds[k][:], AF.Identity,
                                         bias=bias2s[k][:, 0:1],
                                         scale=negdens[k][:, 0:1])
                    t8W = sbq.tile([128, 8], dt.float32, tag="t8W", bufs=4)
                    nc.scalar.activation(t8W[:], m8s[k][:], AF.Identity,
                                         bias=bias2s[k][:, 0:1],
                                         scale=negdens[k][:, 0:1])
                    dWs.append(dW)
                    t8Ws.append(t8W)
                for k, r in enumerate(rr):
                    wrow = sbq.tile([128, s], dt.bfloat16, tag="wrow", bufs=3)
                    nc.vector._custom_dve(MASKED_RECIP, out=wrow[:],
                                          in0=dWs[k][:], s0=t8Ws[k][:, 2:3],
                                          s1=MR_C1, imm2=MR_C2)
                    nc.sync.dma_start(wt[:, :, r * 128:(r + 1) * 128], wrow[:],
                                      transpose=True)

        def emit_interp(b, P, q, wt):
            xI = []
            for m in range(C2C):
                ps_i = psum.tile([128, 512], dt.float32, tag="mmc", bufs=2)
                for j in range(SJ):
                    nc.tensor.matmul(ps_i[:], P["p2T"][:, j, m * 128:(m + 1) * 128],
                                     wt[:, j, :], start=(j == 0),
                                     stop=(j == SJ - 1))
                t = sbq.tile([128, 512], dt.bfloat16, tag=f"xI{m}")
                nc.scalar.copy(t[:], ps_i[:])
                xI.append(t)
            return xI

        def emit_conv0(b, P, q, xI):
            p1r = []
            for m in range(KP):
                t0 = sbq.tile([128, 512], dt.float32, tag="p1l", bufs=3)
                nc.sync.dma_start(t0[:], d_p1[b, m * 128:(m + 1) * 128,
                                               q * 512:(q + 1) * 512])
                p1r.append(t0)
            col = (b * NQ + q) * 512
            for m in range(MC0):
                ps_c = psum.tile([128, 512], dt.float32, tag="mmc", bufs=2)
                nk = KP + 4
                kk = 0
                for ki in range(KP):
                    nc.tensor.matmul(ps_c[:], w0p1T[:, ki, m * 128:(m + 1) * 128],
                                     p1r[ki][:], start=(kk == 0),
                                     stop=(kk == nk - 1))
                    kk += 1
                for g in range(4):
                    nc.tensor.matmul(ps_c[:], w0xiT[:, g, m * 128:(m + 1) * 128],
                                     xI[g % C2C][:], start=(kk == 0),
                                     stop=(kk == nk - 1))
                    kk += 1
                sc = scol(m, b, q)
                nc.scalar.activation(y0st[m][:, col:col + 512], ps_c[:],
                                     AF.Identity, bias=t_b0[:, m:m + 1],
                                     accum_out=s0sum[:, sc:sc + 1])
                nc.vector.scalar_tensor_tensor(
                    out=sqdump[:], in0=y0st[m][:, col:col + 512], scalar=1.0,
                    in1=y0st[m][:, col:col + 512], op0=ALU.mult, op1=ALU.mult,
                    accum_out=s0sq[:, sc:sc + 1])

        preps = {0: emit_prep(0)}
        with tc.tile_pool(name="wsetup", bufs=1) as wsb_pool:
            emit_weight_setup(wsb_pool)
        pending = None       # (b, P, q, wt) whose interp/conv0 are outstanding
        pend_xi = None
        for b in range(Bc):
            for q in range(NQ):
                wt = sbq.tile([128, SJ, 512], dt.bfloat16, tag="wt")
                emit_dist(b, preps[b], q, 0, wt)
                if pending is not None:
                    pend_xi = emit_interp(pending[0], pending[1], pending[2],
                                          pending[3])
                emit_dist(b, preps[b], q, 1, wt)
                if pending is not None:
                    emit_conv0(pending[0], pending[1], pending[2], pend_xi)
                pending = (b, preps[b], q, wt)
                if b == 0 and q == NQ - 1:
                    preps[1] = emit_prep(1)
        pend_xi = emit_interp(*pending)
        emit_conv0(pending[0], pending[1], pending[2], pend_xi)

        # ================= BN stats AllReduce =================
        def bn_allreduce(ssum, ssq, mc, tag):
            loc = consts.tile([128, 2 * mc], dt.float32, tag=f"loc_{tag}")
            for m in range(mc):
                nc.vector.reduce_sum(loc[:, m:m + 1],
                                     ssum[:, m * Bc * NQ:(m + 1) * Bc * NQ],
                                     axis=mybir.AxisListType.X)
                nc.vector.reduce_sum(loc[:, mc + m:mc + m + 1],
                                     ssq[:, m * Bc * NQ:(m + 1) * Bc * NQ],
                                     axis=mybir.AxisListType.X)
            if not use_collective:
                return loc
            dr_in = dram.tile([128, 2 * mc], dt.float32, tag=f"cc_in_{tag}")
            dr_out = dram.tile([128, 2 * mc], dt.float32, tag=f"cc_out_{tag}")
            nc.gpsimd.dma_start(dr_in[:], loc[:])
            nc.gpsimd.collective_compute(
                "AllReduce", ALU.add, replica_groups=replica_groups,
                ins=[dr_in.opt()], outs=[dr_out.opt()])
            glob = consts.tile([128, 2 * mc], dt.float32, tag=f"glob_{tag}")
            nc.sync.dma_start(glob[:], dr_out[:])
            return glob

        def bn_coeffs(glob, mc, t_g, t_be, tag):
            mean = consts.tile([128, mc], dt.float32, tag=f"mean_{tag}")
            nc.vector.tensor_scalar_mul(mean[:], glob[:, 0:mc], 1.0 / NPOS)
            ex2 = consts.tile([128, mc], dt.float32, tag=f"ex2_{tag}")
            nc.vector.tensor_scalar_mul(ex2[:], glob[:, mc:2 * mc], 1.0 / NPOS)
            var = consts.tile([128, mc], dt.float32, tag=f"var_{tag}")
            nc.vector.tensor_mul(var[:], mean[:], mean[:])
            nc.vector.tensor_sub(var[:], ex2[:], var[:])
            std = consts.tile([128, mc], dt.float32, tag=f"std_{tag}")
            nc.vector.tensor_scalar_add(var[:], var[:], BN_EPS)
            nc.scalar.sqrt(std[:], var[:])
            rstd = consts.tile([128, mc], dt.float32, tag=f"rstd_{tag}")
            nc.vector.reciprocal(rstd[:], std[:])
            A = consts.tile([128, mc], dt.float32, tag=f"A_{tag}")
            nc.vector.tensor_mul(A[:], t_g[:], rstd[:])
            Bsh = consts.tile([128, mc], dt.float32, tag=f"B_{tag}")
            nc.vector.tensor_mul(Bsh[:], mean[:], A[:])
            nc.vector.tensor_sub(Bsh[:], t_be[:], Bsh[:])
            return A, Bsh

        glob1 = bn_allreduce(s0sum, s0sq, MC0, "l1")
        A1, B1 = bn_coeffs(glob1, MC0, t_g0, t_be0, "l1")

        # ================= Phase B: BN1+ReLU -> conv1 =================
        QG = 2  # q-group width for the BN apply
        for b in range(Bc):
            for qg in range(0, NQ, QG):
                gcol = (b * NQ + qg) * 512
                a0g = []
                for m in range(MC0):
                    t = sbq.tile([128, QG * 512], dt.bfloat16, tag=f"a0_{m}")
                    nc.scalar.activation(t[:], y0st[m][:, gcol:gcol + QG * 512],
                                         AF.Relu, bias=B1[:, m:m + 1],
                                         scale=A1[:, m:m + 1])
                    a0g.append(t)
                for qi in range(QG):
                    q = qg + qi
                    col = (b * NQ + q) * 512
                    for m in range(MC1):
                        ps_c = psum.tile([128, 512], dt.float32, tag="mmc", bufs=2)
                        for k in range(2 * KC1):
                            nc.tensor.matmul(
                                ps_c[:], w1T[:, k, m * 128:(m + 1) * 128],
                                a0g[k % KC1][:, qi * 512:(qi + 1) * 512],
                                start=(k == 0), stop=(k == 2 * KC1 - 1))
                        sc = scol(m, b, q)
                        nc.scalar.activation(y1st[m][:, col:col + 512], ps_c[:],
                                             AF.Identity, bias=t_b1[:, m:m + 1],
                                             accum_out=s1sum[:, sc:sc + 1])
                        nc.vector.scalar_tensor_tensor(
                            out=sqdump[:], in0=y1st[m][:, col:col + 512],
                            scalar=1.0, in1=y1st[m][:, col:col + 512],
                            op0=ALU.mult, op1=ALU.mult,
                            accum_out=s1sq[:, sc:sc + 1])

        glob2 = bn_allreduce(s1sum, s1sq, MC1, "l2")
        A2, B2 = bn_coeffs(glob2, MC1, t_g1, t_be1, "l2")

        # ================= Phase C: BN2+ReLU -> out =================
        for b in range(Bc):
            for qg in range(0, NQ, QG):
                gcol = (b * NQ + qg) * 512
                for m in range(MC1):
                    t = sbq.tile([128, QG * 512], dt.float32, tag="outt")
                    nc.scalar.activation(t[:], y1st[m][:, gcol:gcol + QG * 512],
                                         AF.Relu, bias=B2[:, m:m + 1],
                                         scale=A2[:, m:m + 1])
                    half = QG * 512 // 2
                    nc.sync.dma_start(d_out[b, m * 128:(m + 1) * 128,
                                            qg * 512:qg * 512 + half],
                                      t[:, 0:half])
                    nc.scalar.dma_start(d_out[b, m * 128:(m + 1) * 128,
                                              qg * 512 + half:(qg + QG) * 512],
                                        t[:, half:QG * 512])

    return nc


_CACHED = {}


def _get_compiled(key, **kw):
    if key not in _CACHED:
        nc = bacc.Bacc()
        build_core_kernel(nc, **kw)
        nc.compile()
        _CACHED[key] = nc
    return _CACHED[key]


def kernel(xyz1, xyz2, points1, points2, w0, b0, g0, be0, w1, b1, g1, be1,
           trace=False):
    xyz1 = np.ascontiguousarray(xyz1, dtype=np.float32)
    xyz2 = np.ascontiguousarray(xyz2, dtype=np.float32)
    points1 = np.ascontiguousarray(points1, dtype=np.float32)
    points2 = np.ascontiguousarray(points2, dtype=np.float32)
    shared = {
        "w0": np.ascontiguousarray(w0, dtype=np.float32),
        "b0": np.ascontiguousarray(b0, dtype=np.float32),
        "g0": np.ascontiguousarray(g0, dtype=np.float32),
        "be0": np.ascontiguousarray(be0, dtype=np.float32),
        "w1": np.ascontiguousarray(w1, dtype=np.float32),
        "b1": np.ascontiguousarray(b1, dtype=np.float32),
        "g1": np.ascontiguousarray(g1, dtype=np.float32),
        "be1": np.ascontiguousarray(be1, dtype=np.float32),
    }
    Bc = B // N_CORES
    nc = _get_compiled("full")
    in_maps = []
    for c in range(N_CORES):
        sl = slice(c * Bc, (c + 1) * Bc)
        in_maps.append({
            "xyz1": xyz1[sl], "xyz2": xyz2[sl],
            "points1": points1[sl], "points2": points2[sl],
            **shared,
        })
    res = run_bass_kernel_spmd(nc, in_maps, core_ids=list(range(N_CORES)),
                               trace=trace)
    out = np.empty((B, M1, N), dtype=np.float32)
    for c in range(N_CORES):
        out[c * Bc:(c + 1) * Bc] = res.results[c]["out"]
    if trace:
        return out, res
    return out
